# revision 7
# baseline (speedup 1.0000x reference)
"""Trainium2 Bass kernel for nn_LocalGatedResidualNetwork.

Pipeline (reference semantics):
  h1 = elu(local_mm(x, W1, b1))          x:[B,L,C] W1:[O1,SIZE,C] -> [B,O1]
  h2 = local_mm(h1, W2, b2)              W2:[O2,SIZE,1]           -> [B,O2]
  g  = (h2@Wl+bl) * sigmoid(h2@Ws+bs)                             -> [B,1]
  y  = BN(pad(x,(0,1)) + g)                                       -> [B,L+1,C]

Strategy: the locally-connected ops are banded dense matmuls.  Define
A1[(t,c), o] = W1[o, t-o, c] (zero outside 0<=t-o<SIZE).  Then
h1[b,o] = sum_{t,c} x[b,t,c] * A1[(t,c),o] -- a dense matmul whose big
operand (A1 = W1 re-banded) streams from HBM exactly once (plus ~20%
band-edge zeros).  Shard o across 8 cores (128 each, O1 padded to 1024);
each core needs only x[:, o0:o0+640, :].  Stage 2 is sharded over the
*contraction* t: each core computes a partial h2 from its local h1 slice
against its A2 band tile; one 32KB AllReduce sums partials.  Stage 3 + the
residual/BN epilogue run redundantly on every core; the output length
(L+1 = 1535, padded 1536) is sharded 8x192 for the writeback.

All host-side prep is layout-only (pad / transpose / strided-view / tile).
"""

import numpy as np

import concourse.bass as bass
import concourse.mybir as mybir
import concourse.tile as tile
from concourse import bacc
from concourse import bass_utils
from concourse.masks import make_identity

# ---- problem constants (hardcoded; kernel.py must be self-contained) ----
B, L, C = 16, 1534, 16
SIZE = 512
O1, O2 = 1023, 512
EPS = 1e-3

NCORES = 8
NO = 128            # stage-1 outputs per core (O1 padded to 1024)
TSPAN = NO + SIZE   # 640 t-positions per core
KCH = TSPAN * C // 128   # 80 K-chunks of 128
NPIECE = 16         # A1/XT split into pieces for DMA/PE overlap
CHPP = KCH // NPIECE     # 5 chunks per piece
LOUT = 192          # output positions per core (8*192 = 1536 >= L+1)
LHI, LLO = 8, 24    # 192 = 8*24 ; partition p = lhi*16 + b ; free f = llo*16 + c

# stage-1 matmul dtype: float32 (exact) or bfloat16 (2x DMA, 4x PE)
ST1_BF16 = True

_F32 = mybir.dt.float32


def _st1_dt():
    return mybir.dt.bfloat16 if ST1_BF16 else mybir.dt.float32


def _st1_np():
    return mybir.dt.np(_st1_dt())


def _stage1_cols(g):
    """Valid o-column range [n0, n0+N) of K-chunk g (band staircase)."""
    if g == 0:
        return 0, 128          # full width: start=True must clear every column
    if g < 64:
        return 0, min(128, 8 * g + 8)
    n0 = 8 * g - 511
    return n0, 128 - n0


# ============================ host-side prep ============================

def _prep_inputs(x, W1, b1, W2, b2, Wl, bl, Ws, bs, gamma, beta, mov_mean, mov_var):
    """Build the 8 per-core input dicts. Layout transforms only."""
    f4 = np.float32
    x = np.ascontiguousarray(x, f4)
    W1 = np.ascontiguousarray(W1, f4)

    # padded W1: rows o in [0,1024), s' = s+128 in [0,768)
    W1p = np.zeros((NO * NCORES, SIZE + 256, C), f4)
    W1p[:O1, 128:128 + SIZE, :] = W1
    sW = W1p.strides  # (o, s, c)

    # padded x on the t axis to 1536 (covers o0+640 for core 7; pad rows = 0)
    xpad = np.zeros((B, NCORES * NO + SIZE, C), f4)
    xpad[:, :L, :] = x

    # W2 band, padded: W2p[o, 512 + s] = W2[o, s]
    W2f = np.ascontiguousarray(W2[:, :, 0], f4)
    W2p = np.zeros((O2, 3 * SIZE), f4)
    W2p[:, SIZE:2 * SIZE] = W2f
    sW2 = W2p.strides

    b1p = np.zeros(NO * NCORES, f4)
    b1p[:O1] = b1

    b2r = np.tile(np.asarray(b2, f4)[None, :], (B, 1))
    wlr = np.tile(np.asarray(Wl[0, :, 0], f4)[None, :], (B, 1))
    wsr = np.tile(np.asarray(Ws[0, :, 0], f4)[None, :], (B, 1))
    blr = np.full((B, 1), float(np.asarray(bl).reshape(-1)[0]), f4)
    bsr = np.full((B, 1), float(np.asarray(bs).reshape(-1)[0]), f4)

    bnp = np.stack([
        np.tile(np.asarray(gamma, f4), LLO),
        np.tile(np.asarray(beta, f4), LLO),
        np.tile(np.asarray(mov_mean, f4), LLO),
        np.tile(np.asarray(mov_var, f4), LLO),
    ])  # [4, 384]

    ebr = np.zeros((B, 128), f4)          # E[b, p] = (p % 16 == b)
    ebr[np.arange(128) % B, np.arange(128)] = 1.0

    # output-stage x slab, padded to 1536 along l (l=1534.. are zeros)
    xop = np.zeros((B, NCORES * LOUT, C), f4)
    xop[:, :L, :] = x

    dtn = _st1_np()
    in_maps = []
    for k in range(NCORES):
        o0 = k * NO
        # A1 band tile [tloc, c, n] = W1p[o0+n, 128 + tloc - n, c]
        a1v = np.lib.stride_tricks.as_strided(
            W1p[o0:, 128:, :],
            shape=(TSPAN, C, NO),
            strides=(sW[1], sW[2], sW[0] - sW[1]),
        )
        a1 = np.ascontiguousarray(a1v).reshape(TSPAN * C, NO).astype(dtn)

        # xT slice [tloc, c, b] = xpad[b, o0+tloc, c]
        xt = np.ascontiguousarray(
            xpad[:, o0:o0 + TSPAN, :].transpose(1, 2, 0)
        ).reshape(TSPAN * C, B).astype(dtn)

        # A2 band tile [tloc, o] = W2p[o, 512 + 128k + tloc - o]
        a2v = np.lib.stride_tricks.as_strided(
            W2p[:, SIZE + NO * k:],
            shape=(NO, O2),
            strides=(sW2[1], sW2[0] - sW2[1]),
        )
        a2 = np.ascontiguousarray(a2v).astype(f4)

        b1r = np.tile(b1p[o0:o0 + NO][None, :], (B, 1)).astype(f4)

        slab = xop[:, k * LOUT:(k + 1) * LOUT, :]          # [B, 192, C]
        xo = np.ascontiguousarray(
            slab.reshape(B, LHI, LLO, C).transpose(1, 0, 2, 3)
        ).reshape(128, LLO * C).astype(f4)

        in_maps.append(dict(
            a1=a1, xt=xt, a2=a2, b1r=b1r, b2r=b2r, wlr=wlr, wsr=wsr,
            blr=blr, bsr=bsr, bnp=bnp, ebr=ebr, xo=xo,
        ))
    return in_maps


def _unshard(results):
    """results: list of 8 dicts with 'y_slice' [128, 384] -> y [B, L+1, C]."""
    parts = []
    for k in range(NCORES):
        ys = np.asarray(results[k]["y_slice"], np.float32)
        parts.append(
            ys.reshape(LHI, B, LLO, C).transpose(1, 0, 2, 3).reshape(B, LOUT, C)
        )
    y = np.concatenate(parts, axis=1)      # [B, 1536, C]
    return np.ascontiguousarray(y[:, :L + 1, :])


# ============================ kernel emission ============================

def emit(tc, ins, outs, rep=0):
    """Emit the per-core program. ins/outs: dicts of bass.AP."""
    nc = tc.nc
    dt1 = _st1_dt()
    add = mybir.AluOpType.add
    mult = mybir.AluOpType.mult
    AF = mybir.ActivationFunctionType

    from contextlib import ExitStack
    with ExitStack() as ctx:
        const = ctx.enter_context(tc.tile_pool(name=f"const{rep}", bufs=1))
        a1p = ctx.enter_context(tc.tile_pool(name=f"a1p{rep}", bufs=NPIECE))
        xtp = ctx.enter_context(tc.tile_pool(name=f"xtp{rep}", bufs=NPIECE))
        work = ctx.enter_context(tc.tile_pool(name=f"work{rep}", bufs=1))
        psum = ctx.enter_context(tc.tile_pool(name=f"psum{rep}", bufs=1, space="PSUM"))
        dram = ctx.enter_context(tc.tile_pool(name=f"dram{rep}", bufs=1, space="DRAM"))
        # ---- constant / small loads ----
        ident16 = const.tile([16, 16], _F32, tag="ident16")
        make_identity(nc, ident16[:])
        ones1 = const.tile([1, 128], _F32, tag="ones1")
        nc.gpsimd.memset(ones1[:], 1.0)
        zb = const.tile([B, 1], _F32, tag="zb")
        nc.gpsimd.memset(zb[:], 0.0)
        eps1 = const.tile([1, 1], _F32, tag="eps1")
        nc.gpsimd.memset(eps1[:], EPS)

        def load(name, shape, dtype=_F32):
            t = const.tile(shape, dtype, tag=name)
            nc.sync.dma_start(t[:], ins[name])
            return t

        A2 = load("a2", [128, 512])
        B1R = load("b1r", [B, 128])
        B2R = load("b2r", [B, 512])
        WLR = load("wlr", [B, 512])
        WSR = load("wsr", [B, 512])
        BLR = load("blr", [B, 1])
        BSR = load("bsr", [B, 1])
        bn_rows = []
        for bi in range(4):
            t = const.tile([1, LLO * C], _F32, tag=f"bn{bi}")
            nc.sync.dma_start(t[:], ins["bnp"][bi:bi + 1, :])
            bn_rows.append(t)
        GAM, BET, MEA, VAR = bn_rows
        EBR = load("ebr", [B, 128])
        XO = load("xo", [128, LLO * C])

        # ---- stage 1: 80 accumulating matmuls over the A1 band ----
        ps1 = psum.tile([B, 128], _F32, tag="ps1")
        a1_dram = ins["a1"]
        xt_dram = ins["xt"]
        pieces = []
        for i in range(NPIECE):
            r0 = i * CHPP * 128
            xt_t = xtp.tile([128, CHPP * B], dt1, tag="xt")
            nc.sync.dma_start(
                xt_t[:].rearrange("p (a b) -> p a b", a=CHPP),
                xt_dram[r0:r0 + CHPP * 128, :].rearrange("(a p) b -> p a b", p=128),
            )
            a1_t = a1p.tile([128, CHPP * 128], dt1, tag="a1")
            nc.sync.dma_start(
                a1_t[:].rearrange("p (a n) -> p a n", a=CHPP),
                a1_dram[r0:r0 + CHPP * 128, :].rearrange("(a p) n -> p a n", p=128),
            )
            pieces.append((xt_t, a1_t))

        for g in range(KCH):
            i, j = divmod(g, CHPP)
            xt_t, a1_t = pieces[i]
            n0, nn = _stage1_cols(g)
            nc.tensor.matmul(
                ps1[:, n0:n0 + nn],
                lhsT=xt_t[:, j * B:(j + 1) * B],
                rhs=a1_t[:, j * 128 + n0:j * 128 + n0 + nn],
                start=(g == 0),
                stop=(g == KCH - 1),
            )

        # ---- stage-1 epilogue: + b1, ELU ----
        z = work.tile([B, 128], _F32, tag="z")
        nc.vector.tensor_add(z[:], ps1[:], B1R[:])
        r = work.tile([B, 128], _F32, tag="r")
        nc.scalar.activation(r[:], z[:], AF.Relu, bias=zb[:])
        m = work.tile([B, 128], _F32, tag="m")
        nc.vector.tensor_sub(m[:], z[:], r[:])          # min(z, 0)
        e = work.tile([B, 128], _F32, tag="e")
        nc.scalar.activation(e[:], m[:], AF.Exp, bias=zb[:])
        h1 = work.tile([B, 128], _F32, tag="h1")
        nc.vector.tensor_add(h1[:], r[:], e[:])
        h1b = work.tile([B, 128], _F32, tag="h1b")
        nc.vector.tensor_scalar_add(h1b[:], h1[:], -1.0)  # elu = relu + exp(min)-1

        # ---- transpose h1 -> [128, B] for the stage-2 contraction ----
        pst = psum.tile([128, B], _F32, tag="pst")
        nc.tensor.transpose(pst[:], h1b[:], ident16[:])
        h1T = work.tile([128, B], _F32, tag="h1T")
        nc.vector.tensor_copy(h1T[:], pst[:])

        # ---- stage 2: partial h2 over local t-range; AllReduce ----
        ps2 = psum.tile([B, 512], _F32, tag="ps2")
        nc.tensor.matmul(ps2[:], lhsT=h1T[:], rhs=A2[:], start=True, stop=True)
        part2 = work.tile([B, 512], _F32, tag="part2")
        nc.vector.tensor_copy(part2[:], ps2[:])

        cin = dram.tile([B, 512], _F32, tag="cin")
        cout = dram.tile([B, 512], _F32, tag="cout")
        nc.sync.dma_start(cin[:], part2[:])
        nc.gpsimd.collective_compute(
            "AllReduce", add,
            replica_groups=[list(range(NCORES))],
            ins=[cin[:].opt()],
            outs=[cout[:].opt()],
        )
        h2s = work.tile([B, 512], _F32, tag="h2s")
        nc.sync.dma_start(h2s[:], cout[:])
        h2 = work.tile([B, 512], _F32, tag="h2")
        nc.vector.tensor_add(h2[:], h2s[:], B2R[:])

        # ---- stage 3: GLU -> g[b] ----
        tl = work.tile([B, 512], _F32, tag="tl")
        nc.vector.tensor_mul(tl[:], h2[:], WLR[:])
        lin0 = work.tile([B, 1], _F32, tag="lin0")
        nc.vector.reduce_sum(lin0[:], tl[:], axis=mybir.AxisListType.X)
        lin = work.tile([B, 1], _F32, tag="lin")
        nc.scalar.activation(lin[:], lin0[:], AF.Identity, bias=BLR[:])
        ts_ = work.tile([B, 512], _F32, tag="ts_")
        nc.vector.tensor_mul(ts_[:], h2[:], WSR[:])
        sg0 = work.tile([B, 1], _F32, tag="sg0")
        nc.vector.reduce_sum(sg0[:], ts_[:], axis=mybir.AxisListType.X)
        sg = work.tile([B, 1], _F32, tag="sg")
        nc.scalar.activation(sg[:], sg0[:], AF.Sigmoid, bias=BSR[:])
        gg = work.tile([B, 1], _F32, tag="gg")
        nc.vector.tensor_mul(gg[:], lin[:], sg[:])

        # broadcast g[b] to 128 partitions (p%16 == b) via E matmul
        psg = psum.tile([128, 1], _F32, tag="psg")
        nc.tensor.matmul(psg[:], lhsT=EBR[:], rhs=gg[:], start=True, stop=True)
        grep = work.tile([128, 1], _F32, tag="grep")
        nc.vector.tensor_copy(grep[:], psg[:])

        # ---- BN affine: scale = gamma*rsqrt(var+eps), shift = beta - mean*scale
        sd = work.tile([1, LLO * C], _F32, tag="sd")
        nc.scalar.activation(sd[:], VAR[:], AF.Sqrt, bias=eps1[:])
        rsd = work.tile([1, LLO * C], _F32, tag="rsd")
        nc.vector.reciprocal(rsd[:], sd[:])
        scale1 = work.tile([1, LLO * C], _F32, tag="scale1")
        nc.vector.tensor_mul(scale1[:], GAM[:], rsd[:])
        ms1 = work.tile([1, LLO * C], _F32, tag="ms1")
        nc.vector.tensor_mul(ms1[:], MEA[:], scale1[:])
        shift1 = work.tile([1, LLO * C], _F32, tag="shift1")
        nc.vector.tensor_sub(shift1[:], BET[:], ms1[:])

        # broadcast [1, 384] -> [128, 384] via ones-matmul
        psS = psum.tile([128, LLO * C], _F32, tag="psS")
        nc.tensor.matmul(psS[:], lhsT=ones1[:], rhs=scale1[:], start=True, stop=True)
        scaleB = work.tile([128, LLO * C], _F32, tag="scaleB")
        nc.vector.tensor_copy(scaleB[:], psS[:])
        psH = psum.tile([128, LLO * C], _F32, tag="psH")
        nc.tensor.matmul(psH[:], lhsT=ones1[:], rhs=shift1[:], start=True, stop=True)
        shiftB = work.tile([128, LLO * C], _F32, tag="shiftB")
        nc.vector.tensor_copy(shiftB[:], psH[:])

        # ---- y = (xo + g) * scale + shift ----
        t1 = work.tile([128, LLO * C], _F32, tag="t1")
        nc.vector.scalar_tensor_tensor(
            t1[:], XO[:], grep[:], scaleB[:], op0=add, op1=mult
        )
        yt = work.tile([128, LLO * C], _F32, tag="yt")
        nc.vector.tensor_add(yt[:], t1[:], shiftB[:])
        nc.sync.dma_start(outs["y_slice"], yt[:])


# ============================ build + run ============================

_CACHE = {}


def _build(repeat=1):
    key = ("nc", ST1_BF16, repeat)
    if key in _CACHE:
        return _CACHE[key]
    nc = bacc.Bacc(
        "TRN2", target_bir_lowering=False, debug=False,
        enable_asserts=False, num_devices=NCORES,
    )
    dt1 = _st1_dt()
    shapes = dict(
        a1=([TSPAN * C, NO], dt1), xt=([TSPAN * C, B], dt1),
        a2=([128, 512], _F32), b1r=([B, 128], _F32), b2r=([B, 512], _F32),
        wlr=([B, 512], _F32), wsr=([B, 512], _F32),
        blr=([B, 1], _F32), bsr=([B, 1], _F32),
        bnp=([4, LLO * C], _F32), ebr=([B, 128], _F32),
        xo=([128, LLO * C], _F32),
    )
    ins = {n: nc.dram_tensor(n, s, d, kind="ExternalInput").ap()
           for n, (s, d) in shapes.items()}
    outs = {"y_slice": nc.dram_tensor(
        "y_slice", [128, LLO * C], _F32, kind="ExternalOutput").ap()}
    with tile.TileContext(nc) as tc:
        for r in range(repeat):
            emit(tc, ins, outs, rep=r)
    nc.compile()
    _CACHE[key] = nc
    return nc


def run_on_hw(in_maps, repeat=1, **kw):
    nc = _build(repeat)
    return bass_utils.run_bass_kernel_spmd(
        nc, in_maps, core_ids=list(range(NCORES)), **kw
    )


def kernel(**inputs):
    inputs = {k: np.asarray(v) for k, v in inputs.items()}
    in_maps = _prep_inputs(**inputs)
    res = run_on_hw(in_maps)
    return _unshard(res.results)


# revision 8
# speedup vs baseline: 5.8829x; 5.8829x over previous
"""Trainium2 Bass kernel for nn_LocalGatedResidualNetwork.

Pipeline (reference semantics):
  h1 = elu(local_mm(x, W1, b1))          x:[B,L,C] W1:[O1,SIZE,C] -> [B,O1]
  h2 = local_mm(h1, W2, b2)              W2:[O2,SIZE,1]           -> [B,O2]
  g  = (h2@Wl+bl) * sigmoid(h2@Ws+bs)                             -> [B,1]
  y  = BN(pad(x,(0,1)) + g)                                       -> [B,L+1,C]

Strategy: the locally-connected ops are banded dense matmuls.  Define
A1[(t,c), o] = W1[o, t-o, c] (zero outside 0<=t-o<SIZE).  Then
h1[b,o] = sum_{t,c} x[b,t,c] * A1[(t,c),o] -- a dense matmul whose big
operand (A1 = W1 re-banded) streams from HBM exactly once (plus ~20%
band-edge zeros).  Shard o across 8 cores (128 each, O1 padded to 1024);
each core needs only x[:, o0:o0+640, :].  Stage 2 is sharded over the
*contraction* t: each core computes a partial h2 from its local h1 slice
against its A2 band tile; one 32KB AllReduce sums partials.  Stage 3 + the
residual/BN epilogue run redundantly on every core; the output length
(L+1 = 1535, padded 1536) is sharded 8x192 for the writeback.

All host-side prep is layout-only (pad / transpose / strided-view / tile).
"""

import numpy as np

import concourse.bass as bass
import concourse.mybir as mybir
import concourse.tile as tile
from concourse import bacc
from concourse import bass_utils
from concourse.masks import make_identity

# ---- problem constants (hardcoded; kernel.py must be self-contained) ----
B, L, C = 16, 1534, 16
SIZE = 512
O1, O2 = 1023, 512
EPS = 1e-3

NCORES = 8
NO = 128            # stage-1 outputs per core (O1 padded to 1024)
TSPAN = NO + SIZE   # 640 t-positions per core
KCH = TSPAN * C // 128   # 80 K-chunks of 128
NPIECE = 16         # A1/XT split into pieces for DMA/PE overlap
CHPP = KCH // NPIECE     # 5 chunks per piece
LOUT = 192          # output positions per core (8*192 = 1536 >= L+1)
LHI, LLO = 8, 24    # 192 = 8*24 ; partition p = lhi*16 + b ; free f = llo*16 + c

# stage-1 matmul dtype: float32 (exact) or bfloat16 (2x DMA, 4x PE)
ST1_BF16 = True

_F32 = mybir.dt.float32


def _st1_dt():
    return mybir.dt.bfloat16 if ST1_BF16 else mybir.dt.float32


def _st1_np():
    return mybir.dt.np(_st1_dt())


def _stage1_cols(g):
    """Valid o-column range [n0, n0+N) of K-chunk g (band staircase)."""
    if g == 0:
        return 0, 128          # full width: start=True must clear every column
    if g < 64:
        return 0, min(128, 8 * g + 8)
    n0 = 8 * g - 511
    return n0, 128 - n0


# ============================ host-side prep ============================

def _prep_inputs(x, W1, b1, W2, b2, Wl, bl, Ws, bs, gamma, beta, mov_mean, mov_var):
    """Build the 8 per-core input dicts. Layout transforms only."""
    f4 = np.float32
    x = np.ascontiguousarray(x, f4)
    W1 = np.ascontiguousarray(W1, f4)

    # padded W1: rows o in [0,1024), s' = s+128 in [0,768)
    W1p = np.zeros((NO * NCORES, SIZE + 256, C), f4)
    W1p[:O1, 128:128 + SIZE, :] = W1
    sW = W1p.strides  # (o, s, c)

    # padded x on the t axis to 1536 (covers o0+640 for core 7; pad rows = 0)
    xpad = np.zeros((B, NCORES * NO + SIZE, C), f4)
    xpad[:, :L, :] = x

    # W2 band, padded: W2p[o, 512 + s] = W2[o, s]
    W2f = np.ascontiguousarray(W2[:, :, 0], f4)
    W2p = np.zeros((O2, 3 * SIZE), f4)
    W2p[:, SIZE:2 * SIZE] = W2f
    sW2 = W2p.strides

    b1p = np.zeros(NO * NCORES, f4)
    b1p[:O1] = b1

    b2r = np.tile(np.asarray(b2, f4)[None, :], (B, 1))
    wlr = np.tile(np.asarray(Wl[0, :, 0], f4)[None, :], (B, 1))
    wsr = np.tile(np.asarray(Ws[0, :, 0], f4)[None, :], (B, 1))
    blr = np.full((B, 1), float(np.asarray(bl).reshape(-1)[0]), f4)
    bsr = np.full((B, 1), float(np.asarray(bs).reshape(-1)[0]), f4)

    bnp = np.stack([
        np.tile(np.asarray(gamma, f4), LLO),
        np.tile(np.asarray(beta, f4), LLO),
        np.tile(np.asarray(mov_mean, f4), LLO),
        np.tile(np.asarray(mov_var, f4), LLO),
    ])  # [4, 384]

    ebr = np.zeros((B, 128), f4)          # E[b, p] = (p % 16 == b)
    ebr[np.arange(128) % B, np.arange(128)] = 1.0

    # output-stage x slab, padded to 1536 along l (l=1534.. are zeros)
    xop = np.zeros((B, NCORES * LOUT, C), f4)
    xop[:, :L, :] = x

    dtn = _st1_np()
    in_maps = []
    for k in range(NCORES):
        o0 = k * NO
        # A1 band tile [tloc, c, n] = W1p[o0+n, 128 + tloc - n, c]
        a1v = np.lib.stride_tricks.as_strided(
            W1p[o0:, 128:, :],
            shape=(TSPAN, C, NO),
            strides=(sW[1], sW[2], sW[0] - sW[1]),
        )
        a1 = np.ascontiguousarray(a1v).reshape(KCH, 128, NO).transpose(1, 0, 2)
        a1 = np.ascontiguousarray(a1).reshape(128, KCH * NO).astype(dtn)

        # xT slice [tloc, c, b] = xpad[b, o0+tloc, c]
        xt = np.ascontiguousarray(
            xpad[:, o0:o0 + TSPAN, :].transpose(1, 2, 0)
        ).reshape(KCH, 128, B).transpose(1, 0, 2)
        xt = np.ascontiguousarray(xt).reshape(128, KCH * B).astype(dtn)

        # A2 band tile [tloc, o] = W2p[o, 512 + 128k + tloc - o]
        a2v = np.lib.stride_tricks.as_strided(
            W2p[:, SIZE + NO * k:],
            shape=(NO, O2),
            strides=(sW2[1], sW2[0] - sW2[1]),
        )
        a2 = np.ascontiguousarray(a2v).astype(f4)

        b1r = np.tile(b1p[o0:o0 + NO][None, :], (B, 1)).astype(f4)

        slab = xop[:, k * LOUT:(k + 1) * LOUT, :]          # [B, 192, C]
        xo = np.ascontiguousarray(
            slab.reshape(B, LHI, LLO, C).transpose(1, 0, 2, 3)
        ).reshape(128, LLO * C).astype(f4)

        in_maps.append(dict(
            a1=a1, xt=xt, a2=a2, b1r=b1r, b2r=b2r, wlr=wlr, wsr=wsr,
            blr=blr, bsr=bsr, bnp=bnp, ebr=ebr, xo=xo,
        ))
    return in_maps


def _unshard(results):
    """results: list of 8 dicts with 'y_slice' [128, 384] -> y [B, L+1, C]."""
    parts = []
    for k in range(NCORES):
        ys = np.asarray(results[k]["y_slice"], np.float32)
        parts.append(
            ys.reshape(LHI, B, LLO, C).transpose(1, 0, 2, 3).reshape(B, LOUT, C)
        )
    y = np.concatenate(parts, axis=1)      # [B, 1536, C]
    return np.ascontiguousarray(y[:, :L + 1, :])


# ============================ kernel emission ============================

def emit(tc, ins, outs, rep=0):
    """Emit the per-core program. ins/outs: dicts of bass.AP."""
    nc = tc.nc
    dt1 = _st1_dt()
    add = mybir.AluOpType.add
    mult = mybir.AluOpType.mult
    AF = mybir.ActivationFunctionType

    from contextlib import ExitStack
    with ExitStack() as ctx:
        const = ctx.enter_context(tc.tile_pool(name=f"const{rep}", bufs=1))
        a1p = ctx.enter_context(tc.tile_pool(name=f"a1p{rep}", bufs=NPIECE))
        xtp = ctx.enter_context(tc.tile_pool(name=f"xtp{rep}", bufs=NPIECE))
        work = ctx.enter_context(tc.tile_pool(name=f"work{rep}", bufs=1))
        psum = ctx.enter_context(tc.tile_pool(name=f"psum{rep}", bufs=1, space="PSUM"))
        dram = ctx.enter_context(tc.tile_pool(name=f"dram{rep}", bufs=1, space="DRAM"))
        # ---- constant / small loads ----
        ident16 = const.tile([16, 16], _F32, tag="ident16")
        make_identity(nc, ident16[:])
        ones1 = const.tile([1, 128], _F32, tag="ones1")
        nc.gpsimd.memset(ones1[:], 1.0)
        zb = const.tile([B, 1], _F32, tag="zb")
        nc.gpsimd.memset(zb[:], 0.0)
        eps1 = const.tile([1, 1], _F32, tag="eps1")
        nc.gpsimd.memset(eps1[:], EPS)

        def load(name, shape, dtype=_F32):
            t = const.tile(shape, dtype, tag=name)
            nc.sync.dma_start(t[:], ins[name])
            return t

        A2 = load("a2", [128, 512])
        B1R = load("b1r", [B, 128])
        B2R = load("b2r", [B, 512])
        WLR = load("wlr", [B, 512])
        WSR = load("wsr", [B, 512])
        BLR = load("blr", [B, 1])
        BSR = load("bsr", [B, 1])
        bn_rows = []
        for bi in range(4):
            t = const.tile([1, LLO * C], _F32, tag=f"bn{bi}")
            nc.sync.dma_start(t[:], ins["bnp"][bi:bi + 1, :])
            bn_rows.append(t)
        GAM, BET, MEA, VAR = bn_rows
        EBR = load("ebr", [B, 128])
        XO = load("xo", [128, LLO * C])

        # ---- stage 1: 80 accumulating matmuls over the A1 band ----
        ps1 = psum.tile([B, 128], _F32, tag="ps1")
        a1_dram = ins["a1"]
        xt_dram = ins["xt"]
        pieces = []
        for i in range(NPIECE):
            xt_t = xtp.tile([128, CHPP * B], dt1, tag="xt")
            nc.sync.dma_start(
                xt_t[:], xt_dram[:, i * CHPP * B:(i + 1) * CHPP * B])
            a1_t = a1p.tile([128, CHPP * 128], dt1, tag="a1")
            nc.sync.dma_start(
                a1_t[:], a1_dram[:, i * CHPP * 128:(i + 1) * CHPP * 128])
            pieces.append((xt_t, a1_t))

        for g in range(KCH):
            i, j = divmod(g, CHPP)
            xt_t, a1_t = pieces[i]
            n0, nn = _stage1_cols(g)
            nc.tensor.matmul(
                ps1[:, n0:n0 + nn],
                lhsT=xt_t[:, j * B:(j + 1) * B],
                rhs=a1_t[:, j * 128 + n0:j * 128 + n0 + nn],
                start=(g == 0),
                stop=(g == KCH - 1),
            )

        # ---- stage-1 epilogue: + b1, ELU ----
        z = work.tile([B, 128], _F32, tag="z")
        nc.vector.tensor_add(z[:], ps1[:], B1R[:])
        r = work.tile([B, 128], _F32, tag="r")
        nc.scalar.activation(r[:], z[:], AF.Relu, bias=zb[:])
        m = work.tile([B, 128], _F32, tag="m")
        nc.vector.tensor_sub(m[:], z[:], r[:])          # min(z, 0)
        e = work.tile([B, 128], _F32, tag="e")
        nc.scalar.activation(e[:], m[:], AF.Exp, bias=zb[:])
        h1 = work.tile([B, 128], _F32, tag="h1")
        nc.vector.tensor_add(h1[:], r[:], e[:])
        h1b = work.tile([B, 128], _F32, tag="h1b")
        nc.vector.tensor_scalar_add(h1b[:], h1[:], -1.0)  # elu = relu + exp(min)-1

        # ---- transpose h1 -> [128, B] for the stage-2 contraction ----
        pst = psum.tile([128, B], _F32, tag="pst")
        nc.tensor.transpose(pst[:], h1b[:], ident16[:])
        h1T = work.tile([128, B], _F32, tag="h1T")
        nc.vector.tensor_copy(h1T[:], pst[:])

        # ---- stage 2: partial h2 over local t-range; AllReduce ----
        ps2 = psum.tile([B, 512], _F32, tag="ps2")
        nc.tensor.matmul(ps2[:], lhsT=h1T[:], rhs=A2[:], start=True, stop=True)
        part2 = work.tile([B, 512], _F32, tag="part2")
        nc.vector.tensor_copy(part2[:], ps2[:])

        cin = dram.tile([B, 512], _F32, tag="cin")
        cout = dram.tile([B, 512], _F32, tag="cout")
        nc.sync.dma_start(cin[:], part2[:])
        nc.gpsimd.collective_compute(
            "AllReduce", add,
            replica_groups=[list(range(NCORES))],
            ins=[cin[:].opt()],
            outs=[cout[:].opt()],
        )
        h2s = work.tile([B, 512], _F32, tag="h2s")
        nc.sync.dma_start(h2s[:], cout[:])
        h2 = work.tile([B, 512], _F32, tag="h2")
        nc.vector.tensor_add(h2[:], h2s[:], B2R[:])

        # ---- stage 3: GLU -> g[b] ----
        tl = work.tile([B, 512], _F32, tag="tl")
        nc.vector.tensor_mul(tl[:], h2[:], WLR[:])
        lin0 = work.tile([B, 1], _F32, tag="lin0")
        nc.vector.reduce_sum(lin0[:], tl[:], axis=mybir.AxisListType.X)
        lin = work.tile([B, 1], _F32, tag="lin")
        nc.scalar.activation(lin[:], lin0[:], AF.Identity, bias=BLR[:])
        ts_ = work.tile([B, 512], _F32, tag="ts_")
        nc.vector.tensor_mul(ts_[:], h2[:], WSR[:])
        sg0 = work.tile([B, 1], _F32, tag="sg0")
        nc.vector.reduce_sum(sg0[:], ts_[:], axis=mybir.AxisListType.X)
        sg = work.tile([B, 1], _F32, tag="sg")
        nc.scalar.activation(sg[:], sg0[:], AF.Sigmoid, bias=BSR[:])
        gg = work.tile([B, 1], _F32, tag="gg")
        nc.vector.tensor_mul(gg[:], lin[:], sg[:])

        # broadcast g[b] to 128 partitions (p%16 == b) via E matmul
        psg = psum.tile([128, 1], _F32, tag="psg")
        nc.tensor.matmul(psg[:], lhsT=EBR[:], rhs=gg[:], start=True, stop=True)
        grep = work.tile([128, 1], _F32, tag="grep")
        nc.vector.tensor_copy(grep[:], psg[:])

        # ---- BN affine: scale = gamma*rsqrt(var+eps), shift = beta - mean*scale
        sd = work.tile([1, LLO * C], _F32, tag="sd")
        nc.scalar.activation(sd[:], VAR[:], AF.Sqrt, bias=eps1[:])
        rsd = work.tile([1, LLO * C], _F32, tag="rsd")
        nc.vector.reciprocal(rsd[:], sd[:])
        scale1 = work.tile([1, LLO * C], _F32, tag="scale1")
        nc.vector.tensor_mul(scale1[:], GAM[:], rsd[:])
        ms1 = work.tile([1, LLO * C], _F32, tag="ms1")
        nc.vector.tensor_mul(ms1[:], MEA[:], scale1[:])
        shift1 = work.tile([1, LLO * C], _F32, tag="shift1")
        nc.vector.tensor_sub(shift1[:], BET[:], ms1[:])

        # broadcast [1, 384] -> [128, 384] via ones-matmul
        psS = psum.tile([128, LLO * C], _F32, tag="psS")
        nc.tensor.matmul(psS[:], lhsT=ones1[:], rhs=scale1[:], start=True, stop=True)
        scaleB = work.tile([128, LLO * C], _F32, tag="scaleB")
        nc.vector.tensor_copy(scaleB[:], psS[:])
        psH = psum.tile([128, LLO * C], _F32, tag="psH")
        nc.tensor.matmul(psH[:], lhsT=ones1[:], rhs=shift1[:], start=True, stop=True)
        shiftB = work.tile([128, LLO * C], _F32, tag="shiftB")
        nc.vector.tensor_copy(shiftB[:], psH[:])

        # ---- y = (xo + g) * scale + shift ----
        t1 = work.tile([128, LLO * C], _F32, tag="t1")
        nc.vector.scalar_tensor_tensor(
            t1[:], XO[:], grep[:], scaleB[:], op0=add, op1=mult
        )
        yt = work.tile([128, LLO * C], _F32, tag="yt")
        nc.vector.tensor_add(yt[:], t1[:], shiftB[:])
        nc.sync.dma_start(outs["y_slice"], yt[:])


# ============================ build + run ============================

_CACHE = {}


def _build(repeat=1):
    key = ("nc", ST1_BF16, repeat)
    if key in _CACHE:
        return _CACHE[key]
    nc = bacc.Bacc(
        "TRN2", target_bir_lowering=False, debug=False,
        enable_asserts=False, num_devices=NCORES,
    )
    dt1 = _st1_dt()
    shapes = dict(
        a1=([128, KCH * NO], dt1), xt=([128, KCH * B], dt1),
        a2=([128, 512], _F32), b1r=([B, 128], _F32), b2r=([B, 512], _F32),
        wlr=([B, 512], _F32), wsr=([B, 512], _F32),
        blr=([B, 1], _F32), bsr=([B, 1], _F32),
        bnp=([4, LLO * C], _F32), ebr=([B, 128], _F32),
        xo=([128, LLO * C], _F32),
    )
    ins = {n: nc.dram_tensor(n, s, d, kind="ExternalInput").ap()
           for n, (s, d) in shapes.items()}
    outs = {"y_slice": nc.dram_tensor(
        "y_slice", [128, LLO * C], _F32, kind="ExternalOutput").ap()}
    with tile.TileContext(nc) as tc:
        for r in range(repeat):
            emit(tc, ins, outs, rep=r)
    nc.compile()
    _CACHE[key] = nc
    return nc


def run_on_hw(in_maps, repeat=1, **kw):
    nc = _build(repeat)
    return bass_utils.run_bass_kernel_spmd(
        nc, in_maps, core_ids=list(range(NCORES)), **kw
    )


def kernel(**inputs):
    inputs = {k: np.asarray(v) for k, v in inputs.items()}
    in_maps = _prep_inputs(**inputs)
    res = run_on_hw(in_maps)
    return _unshard(res.results)


# revision 9
# speedup vs baseline: 178.4971x; 30.3415x over previous
"""Trainium2 Bass kernel for nn_LocalGatedResidualNetwork.

Pipeline (reference semantics):
  h1 = elu(local_mm(x, W1, b1))          x:[B,L,C] W1:[O1,SIZE,C] -> [B,O1]
  h2 = local_mm(h1, W2, b2)              W2:[O2,SIZE,1]           -> [B,O2]
  g  = (h2@Wl+bl) * sigmoid(h2@Ws+bs)                             -> [B,1]
  y  = BN(pad(x,(0,1)) + g)                                       -> [B,L+1,C]

Strategy: the locally-connected ops are banded dense matmuls.  Define
A1[(t,c), o] = W1[o, t-o, c] (zero outside 0<=t-o<SIZE).  Then
h1[b,o] = sum_{t,c} x[b,t,c] * A1[(t,c),o] -- a dense matmul whose big
operand (A1 = W1 re-banded) streams from HBM exactly once (plus ~20%
band-edge zeros).  Shard o across 8 cores (128 each, O1 padded to 1024);
each core needs only x[:, o0:o0+640, :].  Stage 2 is sharded over the
*contraction* t: each core computes a partial h2 from its local h1 slice
against its A2 band tile; one 32KB AllReduce sums partials.  Stage 3 + the
residual/BN epilogue run redundantly on every core; the output length
(L+1 = 1535, padded 1536) is sharded 8x192 for the writeback.

All host-side prep is layout-only (pad / transpose / strided-view / tile).
"""

import numpy as np

import concourse.bass as bass
import concourse.mybir as mybir
import concourse.tile as tile
from concourse import bacc
from concourse import bass_utils
from concourse.masks import make_identity

# ---- problem constants (hardcoded; kernel.py must be self-contained) ----
B, L, C = 16, 1534, 16
SIZE = 512
O1, O2 = 1023, 512
EPS = 1e-3

NCORES = 8
NO = 128            # stage-1 outputs per core (O1 padded to 1024)
TSPAN = NO + SIZE   # 640 t-positions per core
KCH = TSPAN * C // 128   # 80 K-chunks of 128
NPIECE = 16         # A1/XT split into pieces for DMA/PE overlap
CHPP = KCH // NPIECE     # 5 chunks per piece
LOUT = 192          # output positions per core (8*192 = 1536 >= L+1)
LHI, LLO = 8, 24    # 192 = 8*24 ; partition p = lhi*16 + b ; free f = llo*16 + c

# stage-1 matmul dtype: float32 (exact) or bfloat16 (2x DMA, 4x PE)
ST1_BF16 = True
# timing ablation: replace the AllReduce with a dram->dram copy (WRONG results)
NO_COLLECTIVE = False

_F32 = mybir.dt.float32


def _st1_dt():
    return mybir.dt.bfloat16 if ST1_BF16 else mybir.dt.float32


def _st1_np():
    return mybir.dt.np(_st1_dt())


def _stage1_cols(g):
    """Valid o-column range [n0, n0+N) of K-chunk g (band staircase)."""
    if g == 0:
        return 0, 128          # full width: start=True must clear every column
    if g < 64:
        return 0, min(128, 8 * g + 8)
    n0 = 8 * g - 511
    return n0, 128 - n0


# ============================ host-side prep ============================

def _prep_inputs(x, W1, b1, W2, b2, Wl, bl, Ws, bs, gamma, beta, mov_mean, mov_var):
    """Build the 8 per-core input dicts. Layout transforms only."""
    f4 = np.float32
    x = np.ascontiguousarray(x, f4)
    W1 = np.ascontiguousarray(W1, f4)

    # padded W1: rows o in [0,1024), s' = s+128 in [0,768)
    W1p = np.zeros((NO * NCORES, SIZE + 256, C), f4)
    W1p[:O1, 128:128 + SIZE, :] = W1
    sW = W1p.strides  # (o, s, c)

    # padded x on the t axis to 1536 (covers o0+640 for core 7; pad rows = 0)
    xpad = np.zeros((B, NCORES * NO + SIZE, C), f4)
    xpad[:, :L, :] = x

    # W2 band, padded: W2p[o, 512 + s] = W2[o, s]
    W2f = np.ascontiguousarray(W2[:, :, 0], f4)
    W2p = np.zeros((O2, 3 * SIZE), f4)
    W2p[:, SIZE:2 * SIZE] = W2f
    sW2 = W2p.strides

    b1p = np.zeros(NO * NCORES, f4)
    b1p[:O1] = b1

    b2r = np.tile(np.asarray(b2, f4)[None, :], (B, 1))
    wlr = np.tile(np.asarray(Wl[0, :, 0], f4)[None, :], (B, 1))
    wsr = np.tile(np.asarray(Ws[0, :, 0], f4)[None, :], (B, 1))
    blr = np.full((B, 1), float(np.asarray(bl).reshape(-1)[0]), f4)
    bsr = np.full((B, 1), float(np.asarray(bs).reshape(-1)[0]), f4)

    bnp = np.stack([
        np.tile(np.asarray(gamma, f4), LLO),
        np.tile(np.asarray(beta, f4), LLO),
        np.tile(np.asarray(mov_mean, f4), LLO),
        np.tile(np.asarray(mov_var, f4), LLO),
    ])  # [4, 384]

    ebr = np.zeros((B, 128), f4)          # E[b, p] = (p % 16 == b)
    ebr[np.arange(128) % B, np.arange(128)] = 1.0

    # output-stage x slab, padded to 1536 along l (l=1534.. are zeros)
    xop = np.zeros((B, NCORES * LOUT, C), f4)
    xop[:, :L, :] = x

    dtn = _st1_np()
    in_maps = []
    for k in range(NCORES):
        o0 = k * NO
        # A1 band tile [tloc, c, n] = W1p[o0+n, 128 + tloc - n, c]
        a1v = np.lib.stride_tricks.as_strided(
            W1p[o0:, 128:, :],
            shape=(TSPAN, C, NO),
            strides=(sW[1], sW[2], sW[0] - sW[1]),
        )
        a1 = np.ascontiguousarray(a1v).reshape(KCH, 128, NO).transpose(1, 0, 2)
        a1 = np.ascontiguousarray(a1).reshape(128, KCH * NO).astype(dtn)

        # xT slice [tloc, c, b] = xpad[b, o0+tloc, c]
        xt = np.ascontiguousarray(
            xpad[:, o0:o0 + TSPAN, :].transpose(1, 2, 0)
        ).reshape(KCH, 128, B).transpose(1, 0, 2)
        xt = np.ascontiguousarray(xt).reshape(128, KCH * B).astype(dtn)

        # A2 band tile [tloc, o] = W2p[o, 512 + 128k + tloc - o]
        a2v = np.lib.stride_tricks.as_strided(
            W2p[:, SIZE + NO * k:],
            shape=(NO, O2),
            strides=(sW2[1], sW2[0] - sW2[1]),
        )
        a2 = np.ascontiguousarray(a2v).astype(f4)

        b1r = np.tile(b1p[o0:o0 + NO][None, :], (B, 1)).astype(f4)

        slab = xop[:, k * LOUT:(k + 1) * LOUT, :]          # [B, 192, C]
        xo = np.ascontiguousarray(
            slab.reshape(B, LHI, LLO, C).transpose(1, 0, 2, 3)
        ).reshape(128, LLO * C).astype(f4)

        in_maps.append(dict(
            a1=a1, xt=xt, a2=a2, b1r=b1r, b2r=b2r, wlr=wlr, wsr=wsr,
            blr=blr, bsr=bsr, bnp=bnp, ebr=ebr, xo=xo,
        ))
    return in_maps


def _unshard(results):
    """results: list of 8 dicts with 'y_slice' [128, 384] -> y [B, L+1, C]."""
    parts = []
    for k in range(NCORES):
        ys = np.asarray(results[k]["y_slice"], np.float32)
        parts.append(
            ys.reshape(LHI, B, LLO, C).transpose(1, 0, 2, 3).reshape(B, LOUT, C)
        )
    y = np.concatenate(parts, axis=1)      # [B, 1536, C]
    return np.ascontiguousarray(y[:, :L + 1, :])


# ============================ kernel emission ============================

def emit(tc, ins, outs, rep=0):
    """Emit the per-core program. ins/outs: dicts of bass.AP."""
    nc = tc.nc
    dt1 = _st1_dt()
    add = mybir.AluOpType.add
    mult = mybir.AluOpType.mult
    AF = mybir.ActivationFunctionType

    from contextlib import ExitStack
    with ExitStack() as ctx:
        const = ctx.enter_context(tc.tile_pool(name=f"const{rep}", bufs=1))
        a1p = ctx.enter_context(tc.tile_pool(name=f"a1p{rep}", bufs=NPIECE))
        xtp = ctx.enter_context(tc.tile_pool(name=f"xtp{rep}", bufs=NPIECE))
        work = ctx.enter_context(tc.tile_pool(name=f"work{rep}", bufs=1))
        psum = ctx.enter_context(tc.tile_pool(name=f"psum{rep}", bufs=1, space="PSUM"))
        dram = ctx.enter_context(tc.tile_pool(name=f"dram{rep}", bufs=1, space="DRAM"))
        # ---- constant / small loads ----
        ident16 = const.tile([16, 16], _F32, tag="ident16")
        make_identity(nc, ident16[:])
        ones1 = const.tile([1, 128], _F32, tag="ones1")
        nc.gpsimd.memset(ones1[:], 1.0)
        zb = const.tile([B, 1], _F32, tag="zb")
        nc.gpsimd.memset(zb[:], 0.0)
        eps1 = const.tile([1, 1], _F32, tag="eps1")
        nc.gpsimd.memset(eps1[:], EPS)

        def load(name, shape, dtype=_F32):
            t = const.tile(shape, dtype, tag=name)
            nc.sync.dma_start(t[:], ins[name])
            return t

        A2 = load("a2", [128, 512])
        B1R = load("b1r", [B, 128])
        B2R = load("b2r", [B, 512])
        WLR = load("wlr", [B, 512])
        WSR = load("wsr", [B, 512])
        BLR = load("blr", [B, 1])
        BSR = load("bsr", [B, 1])
        bn_rows = []
        for bi in range(4):
            t = const.tile([1, LLO * C], _F32, tag=f"bn{bi}")
            nc.sync.dma_start(t[:], ins["bnp"][bi:bi + 1, :])
            bn_rows.append(t)
        GAM, BET, MEA, VAR = bn_rows
        EBR = load("ebr", [B, 128])
        XO = load("xo", [128, LLO * C])

        # ---- stage 1: 80 accumulating matmuls over the A1 band ----
        ps1 = psum.tile([B, 128], _F32, tag="ps1")
        a1_dram = ins["a1"]
        xt_dram = ins["xt"]
        pieces = []
        for i in range(NPIECE):
            xt_t = xtp.tile([128, CHPP * B], dt1, tag="xt")
            nc.sync.dma_start(
                xt_t[:], xt_dram[:, i * CHPP * B:(i + 1) * CHPP * B])
            a1_t = a1p.tile([128, CHPP * 128], dt1, tag="a1")
            nc.sync.dma_start(
                a1_t[:], a1_dram[:, i * CHPP * 128:(i + 1) * CHPP * 128])
            pieces.append((xt_t, a1_t))

        for g in range(KCH):
            i, j = divmod(g, CHPP)
            xt_t, a1_t = pieces[i]
            n0, nn = _stage1_cols(g)
            nc.tensor.matmul(
                ps1[:, n0:n0 + nn],
                lhsT=xt_t[:, j * B:(j + 1) * B],
                rhs=a1_t[:, j * 128 + n0:j * 128 + n0 + nn],
                start=(g == 0),
                stop=(g == KCH - 1),
            )

        # ---- stage-1 epilogue: + b1, ELU ----
        z = work.tile([B, 128], _F32, tag="z")
        nc.vector.tensor_add(z[:], ps1[:], B1R[:])
        r = work.tile([B, 128], _F32, tag="r")
        nc.scalar.activation(r[:], z[:], AF.Relu, bias=zb[:])
        m = work.tile([B, 128], _F32, tag="m")
        nc.vector.tensor_sub(m[:], z[:], r[:])          # min(z, 0)
        e = work.tile([B, 128], _F32, tag="e")
        nc.scalar.activation(e[:], m[:], AF.Exp, bias=zb[:])
        h1 = work.tile([B, 128], _F32, tag="h1")
        nc.vector.tensor_add(h1[:], r[:], e[:])
        h1b = work.tile([B, 128], _F32, tag="h1b")
        nc.vector.tensor_scalar_add(h1b[:], h1[:], -1.0)  # elu = relu + exp(min)-1

        # ---- transpose h1 -> [128, B] for the stage-2 contraction ----
        pst = psum.tile([128, B], _F32, tag="pst")
        nc.tensor.transpose(pst[:], h1b[:], ident16[:])
        h1T = work.tile([128, B], _F32, tag="h1T")
        nc.vector.tensor_copy(h1T[:], pst[:])

        # ---- stage 2: partial h2 over local t-range; AllReduce ----
        ps2 = psum.tile([B, 512], _F32, tag="ps2")
        nc.tensor.matmul(ps2[:], lhsT=h1T[:], rhs=A2[:], start=True, stop=True)
        part2 = work.tile([B, 512], _F32, tag="part2")
        nc.vector.tensor_copy(part2[:], ps2[:])

        cin = dram.tile([B, 512], _F32, tag="cin")
        cout = dram.tile([B, 512], _F32, tag="cout")
        nc.sync.dma_start(cin[:], part2[:])
        if NO_COLLECTIVE:
            nc.sync.dma_start(cout[:], cin[:])
        else:
            nc.gpsimd.collective_compute(
                "AllReduce", add,
                replica_groups=[list(range(NCORES))],
                ins=[cin[:].opt()],
                outs=[cout[:].opt()],
            )
        h2s = work.tile([B, 512], _F32, tag="h2s")
        nc.sync.dma_start(h2s[:], cout[:])
        h2 = work.tile([B, 512], _F32, tag="h2")
        nc.vector.tensor_add(h2[:], h2s[:], B2R[:])

        # ---- stage 3: GLU -> g[b] ----
        tl = work.tile([B, 512], _F32, tag="tl")
        nc.vector.tensor_mul(tl[:], h2[:], WLR[:])
        lin0 = work.tile([B, 1], _F32, tag="lin0")
        nc.vector.reduce_sum(lin0[:], tl[:], axis=mybir.AxisListType.X)
        lin = work.tile([B, 1], _F32, tag="lin")
        nc.scalar.activation(lin[:], lin0[:], AF.Identity, bias=BLR[:])
        ts_ = work.tile([B, 512], _F32, tag="ts_")
        nc.vector.tensor_mul(ts_[:], h2[:], WSR[:])
        sg0 = work.tile([B, 1], _F32, tag="sg0")
        nc.vector.reduce_sum(sg0[:], ts_[:], axis=mybir.AxisListType.X)
        sg = work.tile([B, 1], _F32, tag="sg")
        nc.scalar.activation(sg[:], sg0[:], AF.Sigmoid, bias=BSR[:])
        gg = work.tile([B, 1], _F32, tag="gg")
        nc.vector.tensor_mul(gg[:], lin[:], sg[:])

        # broadcast g[b] to 128 partitions (p%16 == b) via E matmul
        psg = psum.tile([128, 1], _F32, tag="psg")
        nc.tensor.matmul(psg[:], lhsT=EBR[:], rhs=gg[:], start=True, stop=True)
        grep = work.tile([128, 1], _F32, tag="grep")
        nc.vector.tensor_copy(grep[:], psg[:])

        # ---- BN affine: scale = gamma*rsqrt(var+eps), shift = beta - mean*scale
        sd = work.tile([1, LLO * C], _F32, tag="sd")
        nc.scalar.activation(sd[:], VAR[:], AF.Sqrt, bias=eps1[:])
        rsd = work.tile([1, LLO * C], _F32, tag="rsd")
        nc.vector.reciprocal(rsd[:], sd[:])
        scale1 = work.tile([1, LLO * C], _F32, tag="scale1")
        nc.vector.tensor_mul(scale1[:], GAM[:], rsd[:])
        ms1 = work.tile([1, LLO * C], _F32, tag="ms1")
        nc.vector.tensor_mul(ms1[:], MEA[:], scale1[:])
        shift1 = work.tile([1, LLO * C], _F32, tag="shift1")
        nc.vector.tensor_sub(shift1[:], BET[:], ms1[:])

        # broadcast [1, 384] -> [128, 384] via ones-matmul
        psS = psum.tile([128, LLO * C], _F32, tag="psS")
        nc.tensor.matmul(psS[:], lhsT=ones1[:], rhs=scale1[:], start=True, stop=True)
        scaleB = work.tile([128, LLO * C], _F32, tag="scaleB")
        nc.vector.tensor_copy(scaleB[:], psS[:])
        psH = psum.tile([128, LLO * C], _F32, tag="psH")
        nc.tensor.matmul(psH[:], lhsT=ones1[:], rhs=shift1[:], start=True, stop=True)
        shiftB = work.tile([128, LLO * C], _F32, tag="shiftB")
        nc.vector.tensor_copy(shiftB[:], psH[:])

        # ---- y = (xo + g) * scale + shift ----
        t1 = work.tile([128, LLO * C], _F32, tag="t1")
        nc.vector.scalar_tensor_tensor(
            t1[:], XO[:], grep[:], scaleB[:], op0=add, op1=mult
        )
        yt = work.tile([128, LLO * C], _F32, tag="yt")
        nc.vector.tensor_add(yt[:], t1[:], shiftB[:])
        nc.sync.dma_start(outs["y_slice"], yt[:])


# ============================ build + run ============================

_CACHE = {}


def _build(repeat=1):
    key = ("nc", ST1_BF16, NO_COLLECTIVE, repeat)
    if key in _CACHE:
        return _CACHE[key]
    nc = bacc.Bacc(
        "TRN2", target_bir_lowering=False, debug=False,
        enable_asserts=False, num_devices=NCORES,
    )
    dt1 = _st1_dt()
    shapes = dict(
        a1=([128, KCH * NO], dt1), xt=([128, KCH * B], dt1),
        a2=([128, 512], _F32), b1r=([B, 128], _F32), b2r=([B, 512], _F32),
        wlr=([B, 512], _F32), wsr=([B, 512], _F32),
        blr=([B, 1], _F32), bsr=([B, 1], _F32),
        bnp=([4, LLO * C], _F32), ebr=([B, 128], _F32),
        xo=([128, LLO * C], _F32),
    )
    ins = {n: nc.dram_tensor(n, s, d, kind="ExternalInput").ap()
           for n, (s, d) in shapes.items()}
    outs = {"y_slice": nc.dram_tensor(
        "y_slice", [128, LLO * C], _F32, kind="ExternalOutput").ap()}
    with tile.TileContext(nc) as tc:
        for r in range(repeat):
            emit(tc, ins, outs, rep=r)
    nc.compile()
    _CACHE[key] = nc
    return nc


def run_on_hw(in_maps, repeat=1, **kw):
    nc = _build(repeat)
    return bass_utils.run_bass_kernel_spmd(
        nc, in_maps, core_ids=list(range(NCORES)), **kw
    )


def kernel(**inputs):
    inputs = {k: np.asarray(v) for k, v in inputs.items()}
    in_maps = _prep_inputs(**inputs)
    res = run_on_hw(in_maps)
    return _unshard(res.results)
